# revision 1
# baseline (speedup 1.0000x reference)
"""GNN message-passing kernel for Trainium2 (8 NeuronCores, SPMD).

Computation (see reference):
  h1 = tanh(segsum(x[src] -> dst) @ W1 + b1)        [uses A(xW) = (Ax)W]
  support2 = h1 @ W2                                 (computed in L1 epilogue)
  h2 = tanh(segsum(support2[src] -> dst) + b2)
  ht = logmap0(proj(h2))  (rowwise scale)
  pooled = segment mean over seg_ids, then expmap0/proj (host epilogue)

Sharding: nodes split contiguously over cores (dst-shard). Each core owns
SHARD nodes, processes the edges whose dst is in its shard.

The spmm is a one-hot matmul over 128-edge windows.  Edges are laid out in
a per-core stream ordered (group, chunk, block); a window may span several
dst blocks, so each (window, block) pair gets its own masked one-hot
S[e,slot] = (slotcol[e]==slot) where slotcol is -1 for edges of other
blocks (DVE is_equal vs iota), accumulated into the block's PSUM acc.

Layer 1 needs no gather: the x table is a host input, so the host ships
x pre-gathered in stream order and the kernel streams it sequentially.
Layer 2 gathers support2 rows via gpsimd.dma_gather (int16 idx, tables
chunked to 32768 rows).  The only cross-core exchange is one AllGather of
support2 (bf16).
"""

import math
from contextlib import ExitStack

import numpy as np
import ml_dtypes

import concourse.bass as bass
import concourse.tile as tile
import concourse.bacc as bacc
from concourse import mybir

BF16 = mybir.dt.bfloat16
F32 = mybir.dt.float32
I16 = mybir.dt.int16
AF = mybir.ActivationFunctionType
ALU = mybir.AluOpType

MAXNORM = 1.0 - 1e-5
MIN_SS = 1e-15

SUB = 1024          # gather indices per dma_gather call (hw ring limit)
GRP = 4             # dst blocks (of 128 nodes) per PSUM group
WB = 8              # windows per L1 stream copy / L2 gather call


class Cfg:
    def __init__(self, n_nodes, in_dim, hid, n_seg, n_cores):
        self.N = n_nodes
        self.IN = in_dim
        self.HID = hid
        self.NSEG = n_seg
        self.NC = n_cores
        self.SHARD = n_nodes // n_cores
        assert self.SHARD % 128 == 0
        self.NBLK = self.SHARD // 128
        assert self.NBLK % GRP == 0
        self.NGRP = self.NBLK // GRP
        self.CH = min(32768, n_nodes)
        assert n_nodes % self.CH == 0
        self.NCHUNK = n_nodes // self.CH
        self.NSEGCH = (n_seg + 127) // 128


def _prep_layer(cfg, src, dst, nchunk, chunk_ids=None):
    """Window/entry schedule for one spmm layer, SPMD-uniform across cores.

    Edges are streamed per core in (g, k, b) order; cells are (g, k) padded
    to the max count over cores, rounded up to whole 128-edge windows.

    Returns dict with:
      nw[g][k]        windows per cell
      sched[g]        list over k of list of (wglob, [[ent, b, start, stop]])
      nweff, nent     total windows / entries
      per_core        list of dicts: order (stream pos -> edge id, -1 pad),
                      slotcol [nent, 128] float32
    """
    NC = cfg.NC
    ch = cfg.N // nchunk
    core = dst // cfg.SHARD
    blk = (dst % cfg.SHARD) // 128
    slot = dst % 128
    g_all = blk // GRP
    b_all = blk % GRP
    chunk = chunk_ids if chunk_ids is not None else src // ch

    cnt = np.zeros((NC, cfg.NGRP, nchunk), dtype=np.int64)
    np.add.at(cnt, (core, g_all, chunk), 1)
    nw = (cnt.max(axis=0) + 127) // 128          # [NGRP, nchunk]
    # every group needs >= 1 window (PSUM start)
    empty_g = nw.sum(axis=1) == 0
    nw[empty_g, 0] = 1
    nweff = int(nw.sum())

    cellw0 = np.zeros((cfg.NGRP, nchunk), dtype=np.int64)  # first wglob of cell
    w = 0
    for g in range(cfg.NGRP):
        for k in range(nchunk):
            cellw0[g, k] = w
            w += int(nw[g, k])

    # per-core streams
    order = np.lexsort((b_all, chunk, g_all, core))
    per_core_blk = []   # block of edge at stream pos, -1 pad
    per_core_slot = []
    per_core_ord = []
    TOT = nweff * 128
    for c in range(NC):
        sel = order[core[order] == c]
        sblk = np.full(TOT, -1, dtype=np.int64)
        sslot = np.full(TOT, -1, dtype=np.int64)
        sord = np.full(TOT, -1, dtype=np.int64)
        cg, ck = g_all[sel], chunk[sel]
        ep = 0
        for g in range(cfg.NGRP):
            for k in range(nchunk):
                n = int(cnt[c, g, k])
                pos = int(cellw0[g, k]) * 128
                if n:
                    s = sel[ep:ep + n]
                    sblk[pos:pos + n] = b_all[s]
                    sslot[pos:pos + n] = slot[s]
                    sord[pos:pos + n] = s
                    ep += n
        assert ep == len(sel)
        per_core_blk.append(sblk)
        per_core_slot.append(sslot)
        per_core_ord.append(sord)

    # entries: union over cores of (window, block) touches
    touched = np.zeros((nweff, GRP), dtype=bool)
    for c in range(NC):
        sb = per_core_blk[c].reshape(nweff, 128)
        for b in range(GRP):
            touched[:, b] |= (sb == b).any(axis=1)
    # every (g, b) needs >= 1 entry (PSUM start/stop); force in first window
    for g in range(cfg.NGRP):
        w0 = int(cellw0[g, 0])
        hi = int(cellw0[g + 1, 0]) if g + 1 < cfg.NGRP else nweff
        for b in range(GRP):
            if not touched[w0:hi, b].any():
                touched[w0, b] = True

    # entry ids in (w, b) order + schedule skeleton
    entof = np.full((nweff, GRP), -1, dtype=np.int64)
    nent = 0
    sched = []
    for g in range(cfg.NGRP):
        gs = []
        for k in range(nchunk):
            ks = []
            for lw in range(int(nw[g, k])):
                wg = int(cellw0[g, k]) + lw
                ents = []
                for b in range(GRP):
                    if touched[wg, b]:
                        entof[wg, b] = nent
                        ents.append([nent, b, False, False])
                        nent += 1
                ks.append((wg, ents))
            gs.append(ks)
        sched.append(gs)
    # start/stop flags per (cell, b): each (g, k, b) accumulation is a
    # self-contained PSUM group (flushed to SBUF between chunks)
    for g in range(cfg.NGRP):
        for k in range(nchunk):
            for b in range(GRP):
                ws = [wg for (wg, ents) in sched[g][k] if entof[wg, b] >= 0]
                if not ws:
                    continue
                first, last = ws[0], ws[-1]
                for (wg, ents) in sched[g][k]:
                    for e in ents:
                        if e[1] == b:
                            if wg == first:
                                e[2] = True
                            if wg == last:
                                e[3] = True

    # per-core slotcol tables [nent, 128]
    per_core = []
    ws_nz, bs_nz = np.nonzero(entof >= 0)
    for c in range(NC):
        sb = per_core_blk[c].reshape(nweff, 128)
        ss = per_core_slot[c].reshape(nweff, 128)
        scol = np.full((nent, 128), -1.0, dtype=np.float32)
        for wg, b in zip(ws_nz, bs_nz):
            e = entof[wg, b]
            scol[e] = np.where(sb[wg] == b, ss[wg], -1).astype(np.float32)
        per_core.append({"order": per_core_ord[c], "slotcol": scol})

    return {"nw": nw, "sched": sched, "nweff": nweff, "nent": nent,
            "cellw0": cellw0, "per_core": per_core, "nchunk": nchunk,
            "ch": ch}


NSLICE = 4   # shard slices / staged AllGathers; L2 chunk j <-> slice j


def host_prep(cfg, src, dst):
    src = np.asarray(src).astype(np.int64)
    dst = np.asarray(dst).astype(np.int64)
    l1 = _prep_layer(cfg, src, dst, 1)
    # L2 chunks keyed by shard-slice of src: slice j of every core's shard
    # is exchanged by staged AllGather j, forming table j of 8*slice rows.
    slice_rows = cfg.SHARD // NSLICE
    chunk_ids = (src % cfg.SHARD) // slice_rows
    l2 = _prep_layer(cfg, src, dst, NSLICE, chunk_ids=chunk_ids)
    l2["ch"] = cfg.NC * slice_rows
    # L1 per-core src node ids (stream order) for the host-side pre-gather
    for c in range(cfg.NC):
        sord = l1["per_core"][c]["order"]
        l1["per_core"][c]["srcidx"] = np.where(
            sord >= 0, src[np.maximum(sord, 0)], 0).astype(np.int64)
    # L2 per-core idx16 tables: position within table j is
    # core(src)*slice_rows + src % slice_rows
    for c in range(cfg.NC):
        sord = l2["per_core"][c]["order"]
        s = src[np.maximum(sord, 0)]
        pos = (s // cfg.SHARD) * slice_rows + (s % slice_rows)
        idxpos = np.where(sord >= 0, pos, 0)
        assert idxpos.max() < 32768
        iw = idxpos.astype(np.int16).reshape(-1, 16).T      # [16, TOT/16]
        iw = np.tile(iw, (8, 1)).copy()                      # [128, TOT/16]
        l2["per_core"][c]["idx16"] = iw.astype(np.int16)
    return l1, l2


def build(cfg, l1, l2, debug_taps=False):
    """Build the Bass program. Returns nc."""
    N, IN, HID = cfg.N, cfg.IN, cfg.HID
    NW1, NW2 = l1["nweff"], l2["nweff"]
    NENT1, NENT2 = l1["nent"], l2["nent"]
    TOT2 = NW2 * 128

    nc = bacc.Bacc("TRN2", target_bir_lowering=False)

    # x pre-gathered in L1 stream order, partition-major:
    # xg[p, w*IN:(w+1)*IN] = x[src(stream pos w*128+p)]
    xg_d = nc.dram_tensor("x_gath", [128, NW1 * IN], BF16, kind="ExternalInput")
    slot1_d = nc.dram_tensor("slot1", [128, NENT1], F32, kind="ExternalInput")
    idx2_d = nc.dram_tensor("idx16", [128, TOT2 // 16], I16, kind="ExternalInput")
    slot2_d = nc.dram_tensor("slot2", [128, NENT2], F32, kind="ExternalInput")
    segid_d = nc.dram_tensor("segid", [128, cfg.NBLK], F32, kind="ExternalInput")
    iota_d = nc.dram_tensor("iota128", [128, 128], BF16, kind="ExternalInput")
    iotas_d = nc.dram_tensor("iota_seg", [128, cfg.NSEGCH * 128], F32, kind="ExternalInput")
    ident_d = nc.dram_tensor("ident", [128, 128], BF16, kind="ExternalInput")
    w1_d = nc.dram_tensor("W1", [IN, HID], BF16, kind="ExternalInput")
    w2_d = nc.dram_tensor("W2", [HID, HID], BF16, kind="ExternalInput")
    b1_d = nc.dram_tensor("b1rep", [128, HID], F32, kind="ExternalInput")
    b2_d = nc.dram_tensor("b2rep", [128, HID], F32, kind="ExternalInput")

    slice_rows = cfg.SHARD // NSLICE
    s2_shard = [nc.dram_tensor(f"s2_shard{j}", [slice_rows, HID], BF16)
                for j in range(NSLICE)]
    s2_full = [nc.dram_tensor(f"s2_full{j}", [cfg.NC * slice_rows, HID], BF16,
                              addr_space="Shared")
               for j in range(NSLICE)]
    out_d = nc.dram_tensor("pooled", [cfg.NSEGCH * 128, HID + 1], F32,
                           kind="ExternalOutput")

    KIN = IN // 128   # k-chunks for W1 (2)
    mg2 = int(l2["nw"].max())                      # max windows per cell
    me1 = max(sum(len(ents) for (_, ents) in l1["sched"][g][0])
              for g in range(cfg.NGRP))
    me2 = max(sum(len(ents) for (_, ents) in l2["sched"][g][k])
              for g in range(cfg.NGRP) for k in range(l2["nchunk"]))

    with tile.TileContext(nc) as tc, ExitStack() as ctx:
        const = ctx.enter_context(tc.tile_pool(name="const", bufs=1))
        idxp = ctx.enter_context(tc.tile_pool(name="idxp", bufs=3))
        slotp = ctx.enter_context(tc.tile_pool(name="slotp", bufs=3))
        ebufp = ctx.enter_context(tc.tile_pool(name="ebufp", bufs=3))
        eb2p = ctx.enter_context(tc.tile_pool(name="eb2p", bufs=8))
        sp = ctx.enter_context(tc.tile_pool(name="sp", bufs=4))
        flshp = ctx.enter_context(tc.tile_pool(name="flshp", bufs=3))
        xtp = ctx.enter_context(tc.tile_pool(name="xtp", bufs=4))
        hp = ctx.enter_context(tc.tile_pool(name="hp", bufs=3))
        h2allp = ctx.enter_context(tc.tile_pool(name="h2allp", bufs=1))
        normp = ctx.enter_context(tc.tile_pool(name="normp", bufs=1))
        htp = ctx.enter_context(tc.tile_pool(name="htp", bufs=3))

        saccp = ctx.enter_context(tc.tile_pool(name="saccp", bufs=1))

        ctx_spmm = ctx.enter_context(ExitStack())
        ps_acc = ctx_spmm.enter_context(tc.tile_pool(name="ps_acc", bufs=4, space="PSUM"))
        ps_tr = ctx_spmm.enter_context(tc.tile_pool(name="ps_tr", bufs=2, space="PSUM"))
        ps_h = ctx_spmm.enter_context(tc.tile_pool(name="ps_h", bufs=2, space="PSUM"))

        # ---- constants ----
        iota128 = const.tile([128, 128], BF16)
        nc.sync.dma_start(iota128[:], iota_d[:])
        iotaseg = const.tile([128, cfg.NSEGCH * 128], F32)
        nc.sync.dma_start(iotaseg[:], iotas_d[:])
        ident = const.tile([128, 128], BF16)
        nc.sync.dma_start(ident[:], ident_d[:])
        segid = const.tile([128, cfg.NBLK], F32)
        nc.sync.dma_start(segid[:], segid_d[:])
        w1_sb = [const.tile([128, HID], BF16, tag=f"w1_{k}", name=f"w1_{k}")
                 for k in range(KIN)]
        for k in range(KIN):
            nc.sync.dma_start(w1_sb[k][:], w1_d[k * 128:(k + 1) * 128, :])
        w2_sb = const.tile([128, HID], BF16)
        nc.sync.dma_start(w2_sb[:], w2_d[:])
        b1_sb = const.tile([128, HID], F32)
        nc.sync.dma_start(b1_sb[:], b1_d[:])
        b2_sb = const.tile([128, HID], F32)
        nc.sync.dma_start(b2_sb[:], b2_d[:])

        h2_all = h2allp.tile([128, cfg.NBLK * HID], BF16)
        norms2 = normp.tile([128, cfg.NBLK], F32)
        scale = normp.tile([128, cfg.NBLK], F32)
        na = normp.tile([128, cfg.NBLK], F32)
        nb_t = normp.tile([128, cfg.NBLK], F32)

        def l1_block(g, b, agg_ps):
            nb = g * GRP + b
            # copy PSUM f32 -> SBUF bf16
            ax = flshp.tile([128, IN], BF16, tag="ax1")
            nc.scalar.activation(ax[:], agg_ps, AF.Copy)
            h_ps = ps_h.tile([128, HID], F32, tag="hps", name="h_ps")
            for h in range(KIN):
                t_ps = ps_tr.tile([128, 128], BF16, tag="tps")
                nc.tensor.transpose(t_ps[:], ax[:, h * 128:(h + 1) * 128], ident[:])
                xt = xtp.tile([128, 128], BF16, tag="xt")
                nc.scalar.activation(xt[:], t_ps[:], AF.Copy)
                nc.tensor.matmul(h_ps[:], xt[:], w1_sb[h][:],
                                 start=(h == 0), stop=(h == KIN - 1))
            htmp = hp.tile([128, HID], F32, tag="htmp")
            nc.vector.tensor_add(htmp[:], h_ps[:], b1_sb[:])
            h1b = hp.tile([128, HID], BF16, tag="h1b")
            nc.scalar.activation(h1b[:], htmp[:], AF.Tanh)
            # support2 = h1 @ W2  (transpose h1, then W2 as moving operand)
            t2_ps = ps_tr.tile([128, 128], BF16, tag="tps")
            nc.tensor.transpose(t2_ps[:], h1b[:], ident[:])
            h1t = xtp.tile([128, 128], BF16, tag="xt")
            nc.scalar.activation(h1t[:], t2_ps[:], AF.Copy)
            s2_ps = ps_h.tile([128, HID], F32, tag="hps", name="h_ps")
            nc.tensor.matmul(s2_ps[:], h1t[:], w2_sb[:], start=True, stop=True)
            s2b = hp.tile([128, HID], BF16, tag="s2b")
            nc.scalar.activation(s2b[:], s2_ps[:], AF.Copy)
            j = nb * 128 // slice_rows
            r0 = nb * 128 - j * slice_rows
            nc.sync.dma_start(s2_shard[j][r0:r0 + 128, :], s2b[:])

        def l2_block(g, b, sacc_ap):
            # bias + tanh + row-norm^2 for one finished block
            nb = g * GRP + b
            htmp = hp.tile([128, HID], F32, tag="htmp")
            nc.vector.tensor_add(htmp[:], sacc_ap, b2_sb[:])
            h2b = h2_all[:, nb * HID:(nb + 1) * HID]
            nc.scalar.activation(h2b, htmp[:], AF.Tanh)
            sq = htp.tile([128, HID], F32, tag="sq")
            nc.vector.tensor_mul(sq[:], h2b, h2b)
            nc.vector.tensor_reduce(norms2[:, nb:nb + 1], sq[:],
                                    mybir.AxisListType.X, ALU.add)

        # ---------------- layer 1: stream pre-gathered x ----------------
        for g in range(cfg.NGRP):
            accs = [ps_acc.tile([128, IN], F32, tag="acc", name=f"acc{b}")
                    for b in range(GRP)]
            wins = l1["sched"][g][0]
            gw0 = wins[0][0]
            gnw = len(wins)
            ge0 = None
            gne = 0
            for (_, ents) in wins:
                for e in ents:
                    if ge0 is None:
                        ge0 = e[0]
                    gne += 1
            st_g = slotp.tile([128, me1], F32, tag="st1")
            nc.sync.dma_start(st_g[:, :gne], slot1_d[:, ge0:ge0 + gne])
            for w0 in range(0, gnw, WB):
                nwb = min(WB, gnw - w0)
                eb = ebufp.tile([128, WB * IN], BF16, tag="eb1")
                nc.sync.dma_start(
                    eb[:, :nwb * IN],
                    xg_d[:, (gw0 + w0) * IN:(gw0 + w0 + nwb) * IN])
                for (wg, ents) in wins[w0:w0 + nwb]:
                    j = wg - gw0 - w0
                    for (ent, b, st_f, sp_f) in ents:
                        s_t = sp.tile([128, 128], BF16, tag="s_t")
                        nc.vector.tensor_scalar(
                            s_t[:], iota128[:],
                            st_g[:, ent - ge0:ent - ge0 + 1],
                            None, ALU.is_equal)
                        nc.tensor.matmul(
                            accs[b][:, :IN], s_t[:],
                            eb[:, j * IN:(j + 1) * IN],
                            start=st_f, stop=sp_f)
            for b in range(GRP):
                l1_block(g, b, accs[b][:, :IN])

            # staged exchange: slice j covers groups 8j..8j+7
            if (g + 1) % (cfg.NGRP // NSLICE) == 0:
                j = (g + 1) // (cfg.NGRP // NSLICE) - 1
                nc.gpsimd.collective_compute(
                    "AllGather",
                    ALU.bypass,
                    ins=[s2_shard[j].ap().opt()],
                    outs=[s2_full[j].ap().opt()],
                    replica_groups=[list(range(cfg.NC))],
                )

        # ---------------- layer 2: gather support2, chunk-major ----------
        # PSUM partial accs per (cell, block), flushed into the SBUF
        # accumulator so chunk k's gathers only wait on AllGather k.
        ctx_spmm.close()
        ctx_l2 = ctx.enter_context(ExitStack())
        pacc_pool = ctx_l2.enter_context(
            tc.tile_pool(name="pacc", bufs=6, space="PSUM"))
        sacc = saccp.tile([128, cfg.NBLK * HID], F32)
        # first/last touching chunk per (g, b)
        touch = {}
        for g in range(cfg.NGRP):
            for k in range(l2["nchunk"]):
                for (_, ents) in l2["sched"][g][k]:
                    for (ent, b, _, _) in ents:
                        key = (g, b)
                        if key not in touch:
                            touch[key] = [k, k]
                        touch[key][1] = k

        for k in range(l2["nchunk"]):
            tbl = s2_full[k]
            for g in range(cfg.NGRP):
                wins = l2["sched"][g][k]
                if not wins:
                    continue
                cw0 = wins[0][0]
                cnw = len(wins)
                ge0 = None
                gne = 0
                bset = set()
                for (_, ents) in wins:
                    for e in ents:
                        if ge0 is None:
                            ge0 = e[0]
                        gne += 1
                        bset.add(e[1])
                it_g = idxp.tile([128, mg2 * 8], I16, tag="it2")
                nc.sync.dma_start(
                    it_g[:, :cnw * 8],
                    idx2_d[:, cw0 * 8:(cw0 + cnw) * 8])
                st_g = slotp.tile([128, me2], F32, tag="st2")
                nc.sync.dma_start(st_g[:, :gne], slot2_d[:, ge0:ge0 + gne])
                paccs = {}
                for b in sorted(bset):
                    paccs[b] = pacc_pool.tile([128, 512], F32, tag="pacc",
                                              name=f"pacc{b}")
                for s0 in range(0, cnw, WB):
                    swins = wins[s0:s0 + WB]
                    nidx = len(swins) * 128
                    lw0 = swins[0][0] - cw0
                    eb = eb2p.tile([128, WB * HID], BF16, tag="eb2")
                    nc.gpsimd.dma_gather(
                        out_ap=eb[:, :len(swins) * HID].rearrange(
                            "p (n f) -> p n f", f=HID),
                        in_ap=tbl[:, :],
                        idxs_ap=it_g[:, lw0 * 8:lw0 * 8 + nidx // 16],
                        num_idxs=nidx,
                        num_idxs_reg=nidx,
                        elem_size=HID,
                    )
                    for (wg, ents) in swins:
                        jw = wg - swins[0][0]
                        for (ent, b, st_f, sp_f) in ents:
                            s_t = sp.tile([128, 128], BF16, tag="s_t")
                            nc.vector.tensor_scalar(
                                s_t[:], iota128[:],
                                st_g[:, ent - ge0:ent - ge0 + 1],
                                None, ALU.is_equal)
                            nc.tensor.matmul(
                                paccs[b][:, :HID], s_t[:],
                                eb[:, jw * HID:(jw + 1) * HID],
                                start=st_f, stop=sp_f)
                # flush partials into SBUF accumulator
                for b in sorted(bset):
                    nb = g * GRP + b
                    sl = sacc[:, nb * HID:(nb + 1) * HID]
                    if touch[(g, b)][0] == k:
                        nc.vector.tensor_copy(sl, paccs[b][:, :HID])
                    else:
                        nc.vector.tensor_add(sl, sl, paccs[b][:, :HID])
                    if touch[(g, b)][1] == k:
                        l2_block(g, b, sl)

        # ---------------- logmap scale ----------------
        # norm = sqrt(max(ss, MIN_SS)); nclip = min(norm, MAXNORM)
        nc.vector.tensor_scalar_max(na[:], norms2[:], MIN_SS)
        nc.scalar.activation(nb_t[:], na[:], AF.Sqrt)        # nb_t = norm
        nc.vector.tensor_scalar_min(na[:], nb_t[:], MAXNORM)  # na = nclip
        # artanh(nclip) = 0.5*ln((1+n)/(1-n)); scale = artanh/norm
        one_m = normp.tile([128, cfg.NBLK], F32)
        nc.vector.tensor_scalar(one_m[:], na[:], -1.0, 1.0, ALU.mult, ALU.add)
        one_p = normp.tile([128, cfg.NBLK], F32)
        nc.vector.tensor_scalar_add(one_p[:], na[:], 1.0)
        rcp = normp.tile([128, cfg.NBLK], F32)
        nc.vector.reciprocal(rcp[:], one_m[:])
        rat = normp.tile([128, cfg.NBLK], F32)
        nc.vector.tensor_mul(rat[:], one_p[:], rcp[:])
        lg = normp.tile([128, cfg.NBLK], F32)
        nc.scalar.activation(lg[:], rat[:], AF.Ln)
        nc.vector.tensor_scalar_mul(lg[:], lg[:], 0.5)
        rcpn = normp.tile([128, cfg.NBLK], F32)
        nc.vector.reciprocal(rcpn[:], nb_t[:])
        nc.vector.tensor_mul(scale[:], lg[:], rcpn[:])

        # ---------------- pooling ----------------
        ctx_l2.close()
        ps_pool = ctx.enter_context(
            tc.tile_pool(name="ps_pool", bufs=max(cfg.NSEGCH, 1), space="PSUM"))
        pool_ps = [ps_pool.tile([128, HID + 1], F32, tag="pool", name=f"pool{sc}")
                   for sc in range(cfg.NSEGCH)]
        for nbk in range(cfg.NBLK):
            h2b = h2_all[:, nbk * HID:(nbk + 1) * HID]
            ht = htp.tile([128, HID + 1], BF16, tag="ht")
            nc.vector.tensor_scalar(ht[:, :HID], h2b, scale[:, nbk:nbk + 1],
                                    None, ALU.mult)
            nc.vector.memset(ht[:, HID:HID + 1], 1.0)
            for sc in range(cfg.NSEGCH):
                sg = sp.tile([128, 128], BF16, tag="sg")
                nc.vector.tensor_scalar(
                    sg[:], iotaseg[:, sc * 128:(sc + 1) * 128],
                    segid[:, nbk:nbk + 1], None, ALU.is_equal)
                nc.tensor.matmul(
                    pool_ps[sc][:], sg[:], ht[:],
                    start=(nbk == 0), stop=(nbk == cfg.NBLK - 1))
        for sc in range(cfg.NSEGCH):
            po = htp.tile([128, HID + 1], F32, tag="po")
            nc.vector.tensor_copy(po[:], pool_ps[sc][:])
            nc.sync.dma_start(out_d[sc * 128:(sc + 1) * 128, :], po[:])

    nc.compile()
    return nc


def host_inputs(cfg, x, seg_ids, W1, b1, W2, b2, l1, l2):
    """Per-core in_maps for run_bass_kernel_spmd."""
    N, IN, HID = cfg.N, cfg.IN, cfg.HID
    x_bf16 = np.ascontiguousarray(np.asarray(x, np.float32).astype(ml_dtypes.bfloat16))
    iota128 = np.tile(np.arange(128, dtype=np.float32), (128, 1)).astype(ml_dtypes.bfloat16)
    iotaseg = np.tile(np.arange(cfg.NSEGCH * 128, dtype=np.float32), (128, 1))
    ident = np.eye(128, dtype=np.float32).astype(ml_dtypes.bfloat16)
    w1 = np.ascontiguousarray(np.asarray(W1, np.float32).astype(ml_dtypes.bfloat16))
    w2 = np.ascontiguousarray(np.asarray(W2, np.float32).astype(ml_dtypes.bfloat16))
    b1r = np.tile(np.asarray(b1, np.float32), (128, 1))
    b2r = np.tile(np.asarray(b2, np.float32), (128, 1))
    seg = np.asarray(seg_ids, np.float32)
    NW1 = l1["nweff"]
    maps = []
    for c in range(cfg.NC):
        segc = seg[c * cfg.SHARD:(c + 1) * cfg.SHARD].reshape(cfg.NBLK, 128).T
        # pre-gathered x, partition-major [128, NW1*IN]
        rows = x_bf16[l1["per_core"][c]["srcidx"]]  # pad -> row 0 (masked)
        xg = np.ascontiguousarray(
            rows.reshape(NW1, 128, IN).transpose(1, 0, 2).reshape(128, NW1 * IN))
        maps.append({
            "x_gath": xg,
            "slot1": np.ascontiguousarray(l1["per_core"][c]["slotcol"].T),
            "idx16": l2["per_core"][c]["idx16"],
            "slot2": np.ascontiguousarray(l2["per_core"][c]["slotcol"].T),
            "segid": np.ascontiguousarray(segc),
            "iota128": iota128,
            "iota_seg": np.ascontiguousarray(iotaseg.astype(np.float32)),
            "ident": ident,
            "W1": w1,
            "W2": w2,
            "b1rep": b1r,
            "b2rep": b2r,
        })
    return maps


def host_epilogue(cfg, partials, batch_size, max_comments):
    """partials: list of per-core [NSEGCH*128, HID+1] f32."""
    acc = np.zeros_like(partials[0], dtype=np.float64)
    for p in partials:
        acc += p.astype(np.float64)
    acc = acc.astype(np.float32)
    nseg = cfg.NSEG
    sums = acc[:nseg, :cfg.HID]
    counts = acc[:nseg, cfg.HID]
    agg = sums / np.maximum(counts, 1.0)[:, None]
    # expmap0 then proj
    ss = np.maximum(np.sum(agg * agg, axis=1), MIN_SS).astype(np.float32)
    norm = np.sqrt(ss)
    y = agg * (np.tanh(norm) / norm)[:, None]
    ssy = np.maximum(np.sum(y * y, axis=1), MIN_SS).astype(np.float32)
    ny = np.sqrt(ssy)
    f = np.where(ny > MAXNORM, MAXNORM / ny, 1.0).astype(np.float32)
    y = y * f[:, None]
    return y.reshape(int(batch_size), int(max_comments), cfg.HID)


# ====================================================================
# Harness entry point: kernel(**inputs) -> np.ndarray
# ====================================================================

_CACHE = {}


def kernel(x, src, dst, seg_ids, W1, b1, W2, b2, batch_size, max_comments):
    """Full-input GNN ComEnc kernel on 8 Trainium2 NeuronCores.

    Accepts the unsharded inputs of reference.setup_inputs() and returns
    the full (batch, max_comments, HID) float32 output.
    """
    from concourse.bass_utils import run_bass_kernel_spmd

    x = np.asarray(x, dtype=np.float32)
    src = np.asarray(src).astype(np.int64)
    dst = np.asarray(dst).astype(np.int64)
    seg_ids = np.asarray(seg_ids).astype(np.int64)
    W1 = np.asarray(W1, dtype=np.float32)
    b1 = np.asarray(b1, dtype=np.float32)
    W2 = np.asarray(W2, dtype=np.float32)
    b2 = np.asarray(b2, dtype=np.float32)
    bs = int(np.asarray(batch_size))
    mc = int(np.asarray(max_comments))

    n_nodes, in_dim = x.shape
    hid = W1.shape[1]
    nseg = bs * mc
    n_cores = 8

    cfg = Cfg(n_nodes, in_dim, hid, nseg, n_cores)
    l1, l2 = host_prep(cfg, src, dst)

    key = (n_nodes, in_dim, hid, nseg, l1["nweff"], l1["nent"],
           l2["nweff"], l2["nent"])
    if key in _CACHE:
        nc = _CACHE[key]
    else:
        nc = build(cfg, l1, l2)
        _CACHE.clear()
        _CACHE[key] = nc

    maps = host_inputs(cfg, x, seg_ids, W1, b1, W2, b2, l1, l2)
    res = run_bass_kernel_spmd(nc, maps, core_ids=list(range(n_cores)))
    partials = [r["pooled"] for r in res.results]
    out = host_epilogue(cfg, partials, bs, mc)
    return np.ascontiguousarray(out.astype(np.float32))



# revision 7
# speedup vs baseline: 1.1602x; 1.1602x over previous
"""GNN message-passing kernel for Trainium2 (8 NeuronCores, SPMD) — v2.

Computation (see reference):
  h1 = tanh(A x @ W1 + b1)          [A(xW) = (Ax)W]
  s2 = h1 @ W2
  h2 = tanh(A s2 + b2)
  ht = logmap0(proj(h2))            (rowwise scale)
  pooled = segment mean over seg_ids, then expmap0/proj (host epilogue)

Sharding: nodes split contiguously over cores (dst-shard), SHARD=16384.

v2 structure per core:
  L1  streams host-pregathered x rows (fp8e3, x*8 / W1 per-scaled) PLUS
      host-built fp8 one-hot panels S (one 128-edge window per dst block,
      padded rows zero).  Feature-major spmm: acc^T[feat, slot] +=
      eb_chunk^T @ S  (eb stationary), so the W1/W2 products need no
      transposes: h1T = W1^T accT (Act tanh with per-partition bias),
      s2T = W2^T h1T, one final transpose to node-major s2 rows.
  Exchange: 2-stage bf16 AllGather of s2 (halves of the node space), each
      stage's slices produced by the first/second half of L1 blocks.
  L2  gathers s2 rows from the exchanged tables (4 sub-tables of 32768
      rows for int16 idx; contiguous per-table gather streams of 8x128
      rows per call), builds slot one-hot masks on DVE (is_equal), and
      accumulates acc2^T[feat, slot] per block in PSUM within a stage;
      stage 0 flushes to an SBUF partial, stage 1 combines + bias (one
      scalar_tensor_tensor) -> Act tanh -> norms via PE ones-matmul ->
      logmap scale (batched) -> transpose -> pooled via fp16 seg masks.
"""

import numpy as np
import ml_dtypes
from contextlib import ExitStack

import concourse.bass as bass
import concourse.tile as tile
import concourse.bacc as bacc
from concourse import mybir

BF16 = mybir.dt.bfloat16
FP16 = mybir.dt.float16
FP8 = mybir.dt.float8e3
F32 = mybir.dt.float32
I16 = mybir.dt.int16
AF = mybir.ActivationFunctionType
ALU = mybir.AluOpType

NP_FP8 = ml_dtypes.float8_e3m4
NP_BF16 = ml_dtypes.bfloat16

MAXNORM = 1.0 - 1e-5
MIN_SS = 1e-15
XSCALE = 8.0     # x shipped as x*XSCALE in fp8, W1 shipped as W1/XSCALE

GRP = 4          # dst blocks per L2 PSUM group
WB = 8           # windows per L1 stream chunk / L2 gather call
WIN = 384        # fp8 bytes per L1 window row: 256 eb + 128 panel


class Cfg:
    def __init__(self, n_nodes, in_dim, hid, n_seg, n_cores):
        self.N = n_nodes
        self.IN = in_dim
        self.HID = hid
        self.NSEG = n_seg
        self.NC = n_cores
        self.SHARD = n_nodes // n_cores
        self.NBLK = self.SHARD // 128
        self.NGRP = self.NBLK // GRP
        self.NSEGCH = (n_seg + 127) // 128
        # L2 sub-tables: 4 tables of SUBROWS rows (int16 idx limit)
        self.SUBROWS = 32768
        self.HALF = self.SHARD // 2      # rows per core per stage


def _prep_l1(cfg, src, dst):
    """Per-block windows (1 block per window). Returns nw1[nb], base1[nb],
    per-core srcidx / slot streams (pad: srcidx=-1)."""
    NC, NBLK = cfg.NC, cfg.NBLK
    core = dst // cfg.SHARD
    nb = (dst % cfg.SHARD) // 128
    slot = dst % 128
    cnt = np.zeros((NC, NBLK), dtype=np.int64)
    np.add.at(cnt, (core, nb), 1)
    nw1 = (cnt.max(axis=0) + 127) // 128
    nw1 = np.maximum(nw1, 1)
    base1 = np.concatenate([[0], np.cumsum(nw1)[:-1]])
    NW1 = int(nw1.sum())
    TOT = NW1 * 128
    order = np.lexsort((slot, nb, core))
    per_core = []
    for c in range(NC):
        sel = order[core[order] == c]
        sidx = np.full(TOT, -1, dtype=np.int64)
        sslot = np.full(TOT, -1, dtype=np.int64)
        cb = nb[sel]
        ep = 0
        for b in range(NBLK):
            n = int(cnt[c, b])
            pos = int(base1[b]) * 128
            if n:
                s = sel[ep:ep + n]
                sidx[pos:pos + n] = src[s]
                sslot[pos:pos + n] = slot[s]
                ep += n
        assert ep == len(sel)
        per_core.append({"srcidx": sidx, "slot": sslot})
    return {"nw1": nw1, "base1": base1, "NW1": NW1, "per_core": per_core}


def _prep_l2(cfg, src, dst):
    """Cells (g, t): t = stage*2 + half keyed by src position in the stage
    tensor. Windows per cell padded to max over cores; entries per
    (window, b). Gather streams are contiguous per t."""
    NC, NGRP = cfg.NC, cfg.NGRP
    core = dst // cfg.SHARD
    blk = (dst % cfg.SHARD) // 128
    slot = dst % 128
    g_all = blk // GRP
    b_all = blk % GRP
    sc = src // cfg.SHARD               # src core
    sr = src % cfg.SHARD
    stg = sr // cfg.HALF                # collective stage
    pos_in_stage = sc * cfg.HALF + (sr - stg * cfg.HALF)
    half = pos_in_stage // cfg.SUBROWS
    t_all = stg * 2 + half
    pos_sub = pos_in_stage % cfg.SUBROWS

    cnt = np.zeros((NC, NGRP, 4), dtype=np.int64)
    np.add.at(cnt, (core, g_all, t_all), 1)
    nw2 = (cnt.max(axis=0) + 127) // 128      # [NGRP, 4]

    # entry scaffolding needs every (g, b, stage) to have >= 1 window slot
    # in t=2*stage if the stage has no touched windows; ensure cell exists.
    for g in range(NGRP):
        for s in range(2):
            if nw2[g, 2 * s] == 0 and nw2[g, 2 * s + 1] == 0:
                nw2[g, 2 * s] = 1

    # per-t stream window bases, in consumption order (stage, g, t)
    wbase = np.zeros((NGRP, 4), dtype=np.int64)   # window idx within t-stream
    nwt = np.zeros(4, dtype=np.int64)
    for s in range(2):
        for g in range(NGRP):
            for t in (2 * s, 2 * s + 1):
                wbase[g, t] = nwt[t]
                nwt[t] += nw2[g, t]

    # per-core streams per t
    order = np.lexsort((b_all, g_all, t_all, core))
    per_core = []
    for c in range(NC):
        sel = order[core[order] == c]
        streams_idx = [np.zeros(int(nwt[t]) * 128, dtype=np.int64) for t in range(4)]
        streams_slot = [np.full(int(nwt[t]) * 128, -1, dtype=np.int64) for t in range(4)]
        streams_blk = [np.full(int(nwt[t]) * 128, -1, dtype=np.int64) for t in range(4)]
        ep = 0
        # order within a core: sorted by (t, g, b)
        for t in range(4):
            for g in range(NGRP):
                n = int(cnt[c, g, t])
                if n == 0:
                    continue
                s = sel[ep:ep + n]
                pos = int(wbase[g, t]) * 128
                streams_idx[t][pos:pos + n] = pos_sub[s]
                streams_slot[t][pos:pos + n] = slot[s]
                streams_blk[t][pos:pos + n] = b_all[s]
                ep += n
        assert ep == len(sel)
        per_core.append({"idx": streams_idx, "slot": streams_slot,
                         "blk": streams_blk})

    # entries: union over cores of (t-window, b) touches
    touched = [np.zeros((int(nwt[t]), GRP), dtype=bool) for t in range(4)]
    for c in range(NC):
        for t in range(4):
            sb = per_core[c]["blk"][t].reshape(-1, 128)
            for b in range(GRP):
                touched[t][:, b] |= (sb == b).any(axis=1)
    # force >= 1 entry per (g, b, stage)
    for g in range(NGRP):
        for s in range(2):
            for b in range(GRP):
                any_t = False
                for t in (2 * s, 2 * s + 1):
                    w0, n = int(wbase[g, t]), int(nw2[g, t])
                    if n and touched[t][w0:w0 + n, b].any():
                        any_t = True
                if not any_t:
                    t0 = 2 * s if nw2[g, 2 * s] > 0 else 2 * s + 1
                    touched[t0][int(wbase[g, t0]), b] = True

    # entry ids in consumption order + start/stop per (g, b, stage)
    sched = []   # per stage: list over g of list of (t, wglob, [(ent, b, st, sp)])
    nent = 0
    for s in range(2):
        sg = []
        for g in range(NGRP):
            cellw = []
            went = {}   # b -> list of positions in cellw entries
            for t in (2 * s, 2 * s + 1):
                w0, n = int(wbase[g, t]), int(nw2[g, t])
                for lw in range(n):
                    ents = []
                    for b in range(GRP):
                        if touched[t][w0 + lw, b]:
                            ents.append([nent, b, False, False])
                            went.setdefault(b, []).append((len(cellw), len(ents) - 1))
                            nent += 1
                    cellw.append((t, w0 + lw, ents))
            for b, lst in went.items():
                wi, ei = lst[0]
                cellw[wi][2][ei][2] = True
                wi, ei = lst[-1]
                cellw[wi][2][ei][3] = True
            sg.append(cellw)
        sched.append(sg)

    # per-core slotcol [nent, 128]
    for c in range(NC):
        scol = np.full((nent, 128), -1.0, dtype=np.float32)
        for s in range(2):
            for g in range(NGRP):
                for (t, wg, ents) in sched[s][g]:
                    sb = per_core[c]["blk"][t][wg * 128:(wg + 1) * 128]
                    ss_ = per_core[c]["slot"][t][wg * 128:(wg + 1) * 128]
                    for (ent, b, _, _) in ents:
                        scol[ent] = np.where(sb == b, ss_, -1).astype(np.float32)
        per_core[c]["slotcol"] = scol

    return {"nw2": nw2, "wbase": wbase, "nwt": nwt, "sched": sched,
            "nent": nent, "per_core": per_core}


def host_prep(cfg, src, dst):
    src = np.asarray(src).astype(np.int64)
    dst = np.asarray(dst).astype(np.int64)
    l1 = _prep_l1(cfg, src, dst)
    l2 = _prep_l2(cfg, src, dst)
    return l1, l2


def build(cfg, l1, l2):
    N, IN, HID = cfg.N, cfg.IN, cfg.HID
    NW1 = l1["NW1"]
    nwt = [int(x) for x in l2["nwt"]]
    NENT2 = l2["nent"]

    nc = bacc.Bacc("TRN2", target_bir_lowering=False)

    xs_d = nc.dram_tensor("xs", [128, NW1 * WIN], FP8, kind="ExternalInput")
    idx_d = [nc.dram_tensor(f"idx{t}", [128, max(nwt[t] * 8, 8)], I16,
                            kind="ExternalInput") for t in range(4)]
    slot2_d = nc.dram_tensor("slot2", [128, NENT2], F32, kind="ExternalInput")
    segid_d = nc.dram_tensor("segid", [128, cfg.NBLK], F32, kind="ExternalInput")
    iota_d = nc.dram_tensor("iota128", [128, 128], BF16, kind="ExternalInput")
    iotas_d = nc.dram_tensor("iota_seg", [128, cfg.NSEGCH * 128], FP16, kind="ExternalInput")
    ident_d = nc.dram_tensor("ident", [128, 128], BF16, kind="ExternalInput")
    w1_d = nc.dram_tensor("W1s", [IN, HID], BF16, kind="ExternalInput")
    w2_d = nc.dram_tensor("W2", [HID, HID], BF16, kind="ExternalInput")
    b1_d = nc.dram_tensor("b1col", [128, 1], F32, kind="ExternalInput")
    b2_d = nc.dram_tensor("b2col", [128, 1], F32, kind="ExternalInput")
    ones_d = nc.dram_tensor("onescol", [128, 1], BF16, kind="ExternalInput")

    s2_sh = [nc.dram_tensor(f"s2_sh{s}", [cfg.HALF, HID], BF16) for s in range(2)]
    s2_full = [nc.dram_tensor(f"s2_full{s}", [cfg.NC * cfg.HALF, HID], BF16,
                              addr_space="Shared") for s in range(2)]
    out_d = nc.dram_tensor("pooled", [cfg.NSEGCH * 128, HID + 1], F32,
                           kind="ExternalOutput")

    KIN = IN // 128
    nw1 = [int(x) for x in l1["nw1"]]
    base1 = [int(x) for x in l1["base1"]]

    with tile.TileContext(nc) as tc, ExitStack() as ctx:
        const = ctx.enter_context(tc.tile_pool(name="const", bufs=1))
        xsp = ctx.enter_context(tc.tile_pool(name="xsp", bufs=4))
        sp = ctx.enter_context(tc.tile_pool(name="sp", bufs=4))
        hp = ctx.enter_context(tc.tile_pool(name="hp", bufs=3))
        saccp = ctx.enter_context(tc.tile_pool(name="saccp", bufs=1))
        normp = ctx.enter_context(tc.tile_pool(name="normp", bufs=1))

        # ---- constants ----
        iota128 = const.tile([128, 128], BF16)
        nc.sync.dma_start(iota128[:], iota_d[:])
        iotaseg = const.tile([128, cfg.NSEGCH * 128], FP16)
        nc.sync.dma_start(iotaseg[:], iotas_d[:])
        ident = const.tile([128, 128], BF16)
        nc.sync.dma_start(ident[:], ident_d[:])
        segid = const.tile([128, cfg.NBLK], F32)
        nc.sync.dma_start(segid[:], segid_d[:])
        w1_sb = [const.tile([128, HID], BF16, tag=f"w1_{k}", name=f"w1_{k}")
                 for k in range(KIN)]
        for k in range(KIN):
            nc.sync.dma_start(w1_sb[k][:], w1_d[k * 128:(k + 1) * 128, :])
        w2_sb = const.tile([128, HID], BF16)
        nc.sync.dma_start(w2_sb[:], w2_d[:])
        b1c = const.tile([128, 1], F32)
        nc.sync.dma_start(b1c[:], b1_d[:])
        b2c = const.tile([128, 1], F32)
        nc.sync.dma_start(b2c[:], b2_d[:])
        onesc = const.tile([128, 1], BF16)
        nc.sync.dma_start(onesc[:], ones_d[:])
        slot2 = const.tile([128, NENT2], F32)
        nc.sync.dma_start(slot2[:], slot2_d[:])

        # ================= layer 1 =================
        ctx_l1 = ctx.enter_context(ExitStack())
        ps_acc = ctx_l1.enter_context(tc.tile_pool(name="ps_acc", bufs=3, space="PSUM"))
        ps_h = ctx_l1.enter_context(tc.tile_pool(name="ps_h", bufs=1, space="PSUM"))
        ps_s2 = ctx_l1.enter_context(tc.tile_pool(name="ps_s2", bufs=1, space="PSUM"))
        ps_tr = ctx_l1.enter_context(tc.tile_pool(name="ps_tr", bufs=2, space="PSUM"))

        def l1_block(nb, acc):
            # acc: PSUM [128, 2*128] f32 feature-major (aggT chunks)
            xt = [hp.tile([128, 128], BF16, tag=f"xt{k}", name=f"xt{k}")
                  for k in range(KIN)]
            for k in range(KIN):
                nc.scalar.activation(xt[k][:], acc[:, k * 128:(k + 1) * 128], AF.Copy)
            h_ps = ps_h.tile([128, 128], F32, tag="hps", name="h_ps")
            for k in range(KIN):
                nc.tensor.matmul(h_ps[:], w1_sb[k][:], xt[k][:],
                                 start=(k == 0), stop=(k == KIN - 1))
            h1t = hp.tile([128, 128], BF16, tag="h1t", name="h1t")
            nc.scalar.activation(h1t[:], h_ps[:], AF.Tanh, bias=b1c[:, 0:1])
            s2_ps = ps_s2.tile([128, 128], F32, tag="s2ps", name="s2_ps")
            nc.tensor.matmul(s2_ps[:], w2_sb[:], h1t[:], start=True, stop=True)
            s2t = hp.tile([128, 128], BF16, tag="s2t", name="s2t")
            nc.scalar.activation(s2t[:], s2_ps[:], AF.Copy)
            tr_ps = ps_tr.tile([128, 128], BF16, tag="trps")
            nc.tensor.transpose(tr_ps[:], s2t[:], ident[:])
            s2n = hp.tile([128, 128], BF16, tag="s2n", name="s2n")
            nc.vector.tensor_copy(s2n[:], tr_ps[:])
            s = nb // (cfg.NBLK // 2)
            r0 = (nb % (cfg.NBLK // 2)) * 128
            nc.sync.dma_start(s2_sh[s][r0:r0 + 128, :], s2n[:])

        # stream chunks of WB windows; windows are block-major contiguous
        win_blk = []
        for b in range(cfg.NBLK):
            win_blk += [b] * nw1[b]
        acc_cur = {}
        for w0 in range(0, NW1, WB):
            nwb = min(WB, NW1 - w0)
            eb = xsp.tile([128, WB * WIN], FP8, tag="xs")
            nc.sync.dma_start(eb[:, :nwb * WIN],
                              xs_d[:, w0 * WIN:(w0 + nwb) * WIN])
            for j in range(nwb):
                w = w0 + j
                b = win_blk[w]
                if b not in acc_cur:
                    acc_cur[b] = ps_acc.tile([128, KIN * 128], F32, tag="acc",
                                             name=f"acc")
                acc = acc_cur[b]
                first = (w == base1[b])
                last = (w == base1[b] + nw1[b] - 1)
                panel = eb[:, j * WIN + 256:j * WIN + 384]
                for k in range(KIN):
                    nc.tensor.matmul(
                        acc[:, k * 128:(k + 1) * 128],
                        eb[:, j * WIN + k * 128:j * WIN + (k + 1) * 128],
                        panel, start=first, stop=last)
                if last:
                    l1_block(b, acc)
                    del acc_cur[b]
                    # stage-0 exchange as soon as its half's blocks are done
                    if b == cfg.NBLK // 2 - 1:
                        nc.gpsimd.collective_compute(
                            "AllGather", ALU.bypass,
                            ins=[s2_sh[0].ap().opt()],
                            outs=[s2_full[0].ap().opt()],
                            replica_groups=[list(range(cfg.NC))])
        nc.gpsimd.collective_compute(
            "AllGather", ALU.bypass,
            ins=[s2_sh[1].ap().opt()], outs=[s2_full[1].ap().opt()],
            replica_groups=[list(range(cfg.NC))])

        # ================= layer 2 =================
        ctx_l1.close()
        ctx_l2 = ctx.enter_context(ExitStack())
        pacc_p = ctx_l2.enter_context(tc.tile_pool(name="pacc", bufs=2, space="PSUM"))
        ps_pool = ctx_l2.enter_context(tc.tile_pool(name="ps_pool", bufs=1, space="PSUM"))
        ps_ss = ctx_l2.enter_context(tc.tile_pool(name="ps_ss", bufs=1, space="PSUM"))
        ps_h2 = ctx_l2.enter_context(tc.tile_pool(name="ps_h2", bufs=2, space="PSUM"))
        ebp = [ctx_l2.enter_context(tc.tile_pool(name=f"eb2_{p}", bufs=4))
               for p in range(2)]
        idxp = ctx_l2.enter_context(tc.tile_pool(name="idxp", bufs=4))
        htp = ctx_l2.enter_context(tc.tile_pool(name="htp", bufs=4))

        sacc = saccp.tile([128, cfg.NBLK * 128], F32)
        ss_all = ps_ss.tile([128, cfg.NBLK], F32, name="ss_all")
        scale = normp.tile([128, cfg.NBLK], F32)
        pool_all = ps_pool.tile([128, cfg.NSEGCH * (HID + 1)], F32, name="pool_all")
        pool_ps = [pool_all[:, s * (HID + 1):(s + 1) * (HID + 1)]
                   for s in range(cfg.NSEGCH)]

        # per-t gather stream state: consumption cursor -> call issuing
        tbl_ap = [s2_full[t // 2][(t % 2) * cfg.SUBROWS:(t % 2 + 1) * cfg.SUBROWS, :]
                  for t in range(4)]
        cur_tile = [None] * 4
        cur_w0 = [0] * 4

        def ensure_window(t, wg):
            """Return (tile, offset) for window wg of stream t, issuing its
            gather call (WB windows) if needed."""
            if cur_tile[t] is None or wg >= cur_w0[t] + WB:
                w0 = (wg // WB) * WB
                nwin = min(WB, nwt[t] - w0)
                it = idxp.tile([128, WB * 8], I16, tag="it")
                nc.sync.dma_start(it[:, :nwin * 8],
                                  idx_d[t][:, w0 * 8:(w0 + nwin) * 8])
                eb = ebp[t % 2].tile([128, WB * HID], BF16, tag=f"eb{t % 2}")
                nc.gpsimd.dma_gather(
                    out_ap=eb[:, :nwin * HID].rearrange("p (n f) -> p n f", f=HID),
                    in_ap=tbl_ap[t],
                    idxs_ap=it[:, :nwin * 8],
                    num_idxs=nwin * 128,
                    num_idxs_reg=nwin * 128,
                    elem_size=HID)
                cur_tile[t] = eb
                cur_w0[t] = w0
            return cur_tile[t], wg - cur_w0[t]

        nbatch = 8   # blocks per logmap batch
        h2_pend = []  # (nb, h2tr slice ap)
        h2b_cur = [None]

        def flush_logmap():
            if not h2_pend:
                return
            b0 = h2_pend[0][0]
            nbk = len(h2_pend)
            ss = ss_all[:, b0:b0 + nbk]
            na = normp.tile([128, nbatch], F32, tag="na")
            nc.vector.tensor_scalar_max(na[:, :nbk], ss, MIN_SS)
            nrm = normp.tile([128, nbatch], F32, tag="nrm")
            nc.scalar.activation(nrm[:, :nbk], na[:, :nbk], AF.Sqrt)
            ncl = normp.tile([128, nbatch], F32, tag="ncl")
            nc.vector.tensor_scalar_min(ncl[:, :nbk], nrm[:, :nbk], MAXNORM)
            om = normp.tile([128, nbatch], F32, tag="om")
            nc.vector.tensor_scalar(om[:, :nbk], ncl[:, :nbk], -1.0, 1.0,
                                    ALU.mult, ALU.add)
            op_ = normp.tile([128, nbatch], F32, tag="op")
            nc.vector.tensor_scalar_add(op_[:, :nbk], ncl[:, :nbk], 1.0)
            rc = normp.tile([128, nbatch], F32, tag="rc")
            nc.vector.reciprocal(rc[:, :nbk], om[:, :nbk])
            rat = normp.tile([128, nbatch], F32, tag="rat")
            nc.vector.tensor_mul(rat[:, :nbk], op_[:, :nbk], rc[:, :nbk])
            lg = normp.tile([128, nbatch], F32, tag="lg")
            nc.scalar.activation(lg[:, :nbk], rat[:, :nbk], AF.Ln)
            rcn = normp.tile([128, nbatch], F32, tag="rcn")
            nc.vector.reciprocal(rcn[:, :nbk], nrm[:, :nbk])
            nc.vector.tensor_mul(rcn[:, :nbk], rcn[:, :nbk], lg[:, :nbk])
            nc.vector.tensor_scalar_mul(scale[:, b0:b0 + nbk], rcn[:, :nbk], 0.5)
            # scale ready: finish blocks
            for (nb, h2tr) in h2_pend:
                ht = htp.tile([128, HID + 1], BF16, tag="ht", name="ht")
                nc.vector.tensor_scalar(ht[:, :HID], h2tr,
                                        scale[:, nb:nb + 1], None, ALU.mult)
                nc.vector.memset(ht[:, HID:HID + 1], 1.0)
                sg = sp.tile([128, cfg.NSEGCH * 128], FP16, tag="sg")
                nc.vector.tensor_scalar(sg[:], iotaseg[:], segid[:, nb:nb + 1],
                                        None, ALU.is_equal)
                for s in range(cfg.NSEGCH):
                    nc.tensor.matmul(pool_ps[s], sg[:, s * 128:(s + 1) * 128],
                                     ht[:], start=(nb == 0),
                                     stop=(nb == cfg.NBLK - 1))
            h2_pend.clear()
            h2b_cur[0] = None

        for stage in range(2):
            for g in range(cfg.NGRP):
                cellw = l2["sched"][stage][g]
                pacc = pacc_p.tile([128, GRP * 128], F32, tag="pacc",
                                   name="pacc")
                for (t, wg, ents) in cellw:
                    eb, joff = ensure_window(t, wg)
                    for (ent, b, st_f, sp_f) in ents:
                        m = sp.tile([128, 128], BF16, tag="m2", name="m2")
                        nc.vector.tensor_scalar(m[:], iota128[:],
                                                slot2[:, ent:ent + 1],
                                                None, ALU.is_equal)
                        nc.tensor.matmul(pacc[:, b * 128:(b + 1) * 128],
                                         eb[:, joff * HID:(joff + 1) * HID],
                                         m[:], start=st_f, stop=sp_f)
                for b in range(GRP):
                    nb = g * GRP + b
                    pb = pacc[:, b * 128:(b + 1) * 128]
                    sl = sacc[:, nb * 128:(nb + 1) * 128]
                    if stage == 0:
                        nc.vector.tensor_copy(sl, pb)
                    else:
                        h2pre = hp.tile([128, 128], F32, tag="h2pre", name="h2pre")
                        nc.vector.scalar_tensor_tensor(
                            h2pre[:], pb, b2c[:, 0:1], sl,
                            ALU.add, ALU.add)
                        h2t = hp.tile([128, 128], BF16, tag="h2t")
                        nc.scalar.activation(h2t[:], h2pre[:], AF.Tanh)
                        sq = hp.tile([128, 128], BF16, tag="sq")
                        nc.scalar.activation(sq[:], h2t[:], AF.Square)
                        nc.tensor.matmul(ss_all[:, nb:nb + 1], sq[:], onesc[:],
                                         start=True, stop=True)
                        if h2b_cur[0] is None:
                            h2b_cur[0] = ps_h2.tile([128, nbatch * 128], BF16,
                                                    tag="h2b", name="h2b")
                        h2tr = h2b_cur[0][:, len(h2_pend) * 128:
                                          (len(h2_pend) + 1) * 128]
                        nc.tensor.transpose(h2tr, h2t[:], ident[:])
                        h2_pend.append((nb, h2tr))
                        if len(h2_pend) >= nbatch:
                            flush_logmap()
        flush_logmap()

        for s in range(cfg.NSEGCH):
            po = htp.tile([128, HID + 1], F32, tag="po")
            nc.vector.tensor_copy(po[:], pool_ps[s])
            nc.sync.dma_start(out_d[s * 128:(s + 1) * 128, :], po[:])

    nc.compile()
    return nc


def host_inputs(cfg, x, seg_ids, W1, b1, W2, b2, l1, l2):
    N, IN, HID = cfg.N, cfg.IN, cfg.HID
    NW1 = l1["NW1"]
    x8 = np.ascontiguousarray((np.asarray(x, np.float32) * XSCALE).astype(NP_FP8))
    iota128 = np.tile(np.arange(128, dtype=np.float32), (128, 1)).astype(NP_BF16)
    iotaseg = np.tile(np.arange(cfg.NSEGCH * 128, dtype=np.float32),
                      (128, 1)).astype(np.float16)
    ident = np.eye(128, dtype=np.float32).astype(NP_BF16)
    w1 = np.ascontiguousarray(
        (np.asarray(W1, np.float32) / XSCALE).astype(NP_BF16))
    w2 = np.ascontiguousarray(np.asarray(W2, np.float32).astype(NP_BF16))
    b1c = np.asarray(b1, np.float32).reshape(128, 1)
    b2c = np.asarray(b2, np.float32).reshape(128, 1)
    ones = np.ones((128, 1), np.float32).astype(NP_BF16)
    seg = np.asarray(seg_ids, np.float32)
    eye128 = np.eye(128, dtype=np.float32)

    maps = []
    for c in range(cfg.NC):
        pc1 = l1["per_core"][c]
        sidx = pc1["srcidx"]
        rows = x8[np.maximum(sidx, 0)]                     # [NW1*128, 256]
        rows[sidx < 0] = 0
        pslot = pc1["slot"]
        panels = np.zeros((NW1 * 128, 128), dtype=NP_FP8)
        valid = pslot >= 0
        panels[np.nonzero(valid)[0], pslot[valid]] = 1.0
        xs = np.concatenate([rows, panels], axis=1)        # [NW1*128, 384]
        xs = np.ascontiguousarray(
            xs.reshape(NW1, 128, WIN).transpose(1, 0, 2).reshape(128, NW1 * WIN))

        pc2 = l2["per_core"][c]
        idxs = {}
        for t in range(4):
            ids = pc2["idx"][t]
            iw = ids.astype(np.int16).reshape(-1, 16).T
            iw = np.tile(iw, (8, 1)).copy()
            idxs[f"idx{t}"] = iw.astype(np.int16)
            if iw.shape[1] == 0:
                idxs[f"idx{t}"] = np.zeros((128, 8), dtype=np.int16)

        segc = seg[c * cfg.SHARD:(c + 1) * cfg.SHARD].reshape(cfg.NBLK, 128).T
        maps.append({
            "xs": xs,
            **idxs,
            "slot2": np.ascontiguousarray(pc2["slotcol"].T),
            "segid": np.ascontiguousarray(segc.astype(np.float32)),
            "iota128": iota128,
            "iota_seg": iotaseg,
            "ident": ident,
            "W1s": w1,
            "W2": w2,
            "b1col": b1c,
            "b2col": b2c,
            "onescol": ones,
        })
    return maps


def host_epilogue(cfg, partials, batch_size, max_comments):
    acc = np.zeros_like(partials[0], dtype=np.float64)
    for p in partials:
        acc += p.astype(np.float64)
    acc = acc.astype(np.float32)
    nseg = cfg.NSEG
    sums = acc[:nseg, :cfg.HID]
    counts = acc[:nseg, cfg.HID]
    agg = sums / np.maximum(counts, 1.0)[:, None]
    ss = np.maximum(np.sum(agg * agg, axis=1), MIN_SS).astype(np.float32)
    norm = np.sqrt(ss)
    y = agg * (np.tanh(norm) / norm)[:, None]
    ssy = np.maximum(np.sum(y * y, axis=1), MIN_SS).astype(np.float32)
    ny = np.sqrt(ssy)
    f = np.where(ny > MAXNORM, MAXNORM / ny, 1.0).astype(np.float32)
    y = y * f[:, None]
    return y.reshape(int(batch_size), int(max_comments), cfg.HID)


# ====================================================================
# Harness entry point
# ====================================================================

_CACHE = {}


def kernel(x, src, dst, seg_ids, W1, b1, W2, b2, batch_size, max_comments):
    """Full-input GNN ComEnc kernel on 8 Trainium2 NeuronCores."""
    from concourse.bass_utils import run_bass_kernel_spmd

    x = np.asarray(x, dtype=np.float32)
    src = np.asarray(src).astype(np.int64)
    dst = np.asarray(dst).astype(np.int64)
    seg_ids = np.asarray(seg_ids).astype(np.int64)
    W1 = np.asarray(W1, dtype=np.float32)
    b1 = np.asarray(b1, dtype=np.float32)
    W2 = np.asarray(W2, dtype=np.float32)
    b2 = np.asarray(b2, dtype=np.float32)
    bs = int(np.asarray(batch_size))
    mc = int(np.asarray(max_comments))

    n_nodes, in_dim = x.shape
    hid = W1.shape[1]
    nseg = bs * mc
    n_cores = 8

    cfg = Cfg(n_nodes, in_dim, hid, nseg, n_cores)
    l1, l2 = host_prep(cfg, src, dst)

    key = (n_nodes, in_dim, hid, nseg, l1["NW1"], l2["nent"],
           tuple(int(v) for v in l2["nwt"]))
    if key in _CACHE:
        nc = _CACHE[key]
    else:
        nc = build(cfg, l1, l2)
        _CACHE.clear()
        _CACHE[key] = nc

    maps = host_inputs(cfg, x, seg_ids, W1, b1, W2, b2, l1, l2)
    res = run_bass_kernel_spmd(nc, maps, core_ids=list(range(n_cores)))
    partials = [r["pooled"] for r in res.results]
    out = host_epilogue(cfg, partials, bs, mc)
    return np.ascontiguousarray(out.astype(np.float32))


# revision 10
# speedup vs baseline: 1.2119x; 1.0446x over previous
"""GNN message-passing kernel for Trainium2 (8 NeuronCores, SPMD) — v2.

Computation (see reference):
  h1 = tanh(A x @ W1 + b1)          [A(xW) = (Ax)W]
  s2 = h1 @ W2
  h2 = tanh(A s2 + b2)
  ht = logmap0(proj(h2))            (rowwise scale)
  pooled = segment mean over seg_ids, then expmap0/proj (host epilogue)

Sharding: nodes split contiguously over cores (dst-shard), SHARD=16384.

v2 structure per core:
  L1  streams host-pregathered x rows (fp8e3, x*8 / W1 per-scaled) PLUS
      host-built fp8 one-hot panels S (one 128-edge window per dst block,
      padded rows zero).  Feature-major spmm: acc^T[feat, slot] +=
      eb_chunk^T @ S  (eb stationary), so the W1/W2 products need no
      transposes: h1T = W1^T accT (Act tanh with per-partition bias),
      s2T = W2^T h1T, one final transpose to node-major s2 rows.
  Exchange: 2-stage bf16 AllGather of s2 (halves of the node space), each
      stage's slices produced by the first/second half of L1 blocks.
  L2  gathers s2 rows from the exchanged tables (4 sub-tables of 32768
      rows for int16 idx; contiguous per-table gather streams of 8x128
      rows per call), builds slot one-hot masks on DVE (is_equal), and
      accumulates acc2^T[feat, slot] per block in PSUM within a stage;
      stage 0 flushes to an SBUF partial, stage 1 combines + bias (one
      scalar_tensor_tensor) -> Act tanh -> norms via PE ones-matmul ->
      logmap scale (batched) -> transpose -> pooled via fp16 seg masks.
"""

import numpy as np
import ml_dtypes
from contextlib import ExitStack

import concourse.bass as bass
import concourse.tile as tile
import concourse.bacc as bacc
from concourse import mybir

BF16 = mybir.dt.bfloat16
FP16 = mybir.dt.float16
FP8 = mybir.dt.float8e3
F32 = mybir.dt.float32
I16 = mybir.dt.int16
AF = mybir.ActivationFunctionType
ALU = mybir.AluOpType

NP_FP8 = ml_dtypes.float8_e3m4
NP_BF16 = ml_dtypes.bfloat16

MAXNORM = 1.0 - 1e-5
MIN_SS = 1e-15
XSCALE = 8.0     # x shipped as x*XSCALE in fp8, W1 shipped as W1/XSCALE

GRP = 4          # dst blocks per L2 PSUM group
WB = 8           # windows per L2 gather call (1024-idx hw ring limit)
WB1 = 32         # windows per L1 stream DMA
IB = 8           # gather calls per idx DMA
SB = 4           # s2 blocks per spill DMA
WIN = 384        # fp8 bytes per L1 window row: 256 eb + 128 panel


class Cfg:
    def __init__(self, n_nodes, in_dim, hid, n_seg, n_cores):
        self.N = n_nodes
        self.IN = in_dim
        self.HID = hid
        self.NSEG = n_seg
        self.NC = n_cores
        self.SHARD = n_nodes // n_cores
        self.NBLK = self.SHARD // 128
        self.NGRP = self.NBLK // GRP
        self.NSEGCH = (n_seg + 127) // 128
        # L2 sub-tables: 4 tables of SUBROWS rows (int16 idx limit)
        self.SUBROWS = 32768
        self.HALF = self.SHARD // 2      # rows per core per stage


def _prep_l1(cfg, src, dst):
    """Per-block windows (1 block per window). Returns nw1[nb], base1[nb],
    per-core srcidx / slot streams (pad: srcidx=-1)."""
    NC, NBLK = cfg.NC, cfg.NBLK
    core = dst // cfg.SHARD
    nb = (dst % cfg.SHARD) // 128
    slot = dst % 128
    cnt = np.zeros((NC, NBLK), dtype=np.int64)
    np.add.at(cnt, (core, nb), 1)
    nw1 = (cnt.max(axis=0) + 127) // 128
    nw1 = np.maximum(nw1, 1)
    base1 = np.concatenate([[0], np.cumsum(nw1)[:-1]])
    NW1 = int(nw1.sum())
    TOT = NW1 * 128
    order = np.lexsort((slot, nb, core))
    per_core = []
    for c in range(NC):
        sel = order[core[order] == c]
        sidx = np.full(TOT, -1, dtype=np.int64)
        sslot = np.full(TOT, -1, dtype=np.int64)
        cb = nb[sel]
        ep = 0
        for b in range(NBLK):
            n = int(cnt[c, b])
            pos = int(base1[b]) * 128
            if n:
                s = sel[ep:ep + n]
                sidx[pos:pos + n] = src[s]
                sslot[pos:pos + n] = slot[s]
                ep += n
        assert ep == len(sel)
        per_core.append({"srcidx": sidx, "slot": sslot})
    return {"nw1": nw1, "base1": base1, "NW1": NW1, "per_core": per_core}


def _prep_l2(cfg, src, dst):
    """Cells (g, t): t = stage*2 + half keyed by src position in the stage
    tensor. Windows per cell padded to max over cores; entries per
    (window, b). Gather streams are contiguous per t."""
    NC, NGRP = cfg.NC, cfg.NGRP
    core = dst // cfg.SHARD
    blk = (dst % cfg.SHARD) // 128
    slot = dst % 128
    g_all = blk // GRP
    b_all = blk % GRP
    sc = src // cfg.SHARD               # src core
    sr = src % cfg.SHARD
    stg = sr // cfg.HALF                # collective stage
    pos_in_stage = sc * cfg.HALF + (sr - stg * cfg.HALF)
    half = pos_in_stage // cfg.SUBROWS
    t_all = stg * 2 + half
    pos_sub = pos_in_stage % cfg.SUBROWS

    cnt = np.zeros((NC, NGRP, 4), dtype=np.int64)
    np.add.at(cnt, (core, g_all, t_all), 1)
    nw2 = (cnt.max(axis=0) + 127) // 128      # [NGRP, 4]

    # entry scaffolding needs every (g, b, stage) to have >= 1 window slot
    # in t=2*stage if the stage has no touched windows; ensure cell exists.
    for g in range(NGRP):
        for s in range(2):
            if nw2[g, 2 * s] == 0 and nw2[g, 2 * s + 1] == 0:
                nw2[g, 2 * s] = 1

    # per-t stream window bases, in consumption order (stage, g, t)
    wbase = np.zeros((NGRP, 4), dtype=np.int64)   # window idx within t-stream
    nwt = np.zeros(4, dtype=np.int64)
    for s in range(2):
        for g in range(NGRP):
            for t in (2 * s, 2 * s + 1):
                wbase[g, t] = nwt[t]
                nwt[t] += nw2[g, t]

    # per-core streams per t
    order = np.lexsort((b_all, g_all, t_all, core))
    per_core = []
    for c in range(NC):
        sel = order[core[order] == c]
        streams_idx = [np.zeros(int(nwt[t]) * 128, dtype=np.int64) for t in range(4)]
        streams_slot = [np.full(int(nwt[t]) * 128, -1, dtype=np.int64) for t in range(4)]
        streams_blk = [np.full(int(nwt[t]) * 128, -1, dtype=np.int64) for t in range(4)]
        ep = 0
        # order within a core: sorted by (t, g, b)
        for t in range(4):
            for g in range(NGRP):
                n = int(cnt[c, g, t])
                if n == 0:
                    continue
                s = sel[ep:ep + n]
                pos = int(wbase[g, t]) * 128
                streams_idx[t][pos:pos + n] = pos_sub[s]
                streams_slot[t][pos:pos + n] = slot[s]
                streams_blk[t][pos:pos + n] = b_all[s]
                ep += n
        assert ep == len(sel)
        per_core.append({"idx": streams_idx, "slot": streams_slot,
                         "blk": streams_blk})

    # entries: union over cores of (t-window, b) touches
    touched = [np.zeros((int(nwt[t]), GRP), dtype=bool) for t in range(4)]
    for c in range(NC):
        for t in range(4):
            sb = per_core[c]["blk"][t].reshape(-1, 128)
            for b in range(GRP):
                touched[t][:, b] |= (sb == b).any(axis=1)
    # force >= 1 entry per (g, b, stage)
    for g in range(NGRP):
        for s in range(2):
            for b in range(GRP):
                any_t = False
                for t in (2 * s, 2 * s + 1):
                    w0, n = int(wbase[g, t]), int(nw2[g, t])
                    if n and touched[t][w0:w0 + n, b].any():
                        any_t = True
                if not any_t:
                    t0 = 2 * s if nw2[g, 2 * s] > 0 else 2 * s + 1
                    touched[t0][int(wbase[g, t0]), b] = True

    # entry ids in consumption order + start/stop per (g, b, stage)
    sched = []   # per stage: list over g of list of (t, wglob, [(ent, b, st, sp)])
    nent = 0
    for s in range(2):
        sg = []
        for g in range(NGRP):
            cellw = []
            went = {}   # b -> list of positions in cellw entries
            for t in (2 * s, 2 * s + 1):
                w0, n = int(wbase[g, t]), int(nw2[g, t])
                for lw in range(n):
                    ents = []
                    for b in range(GRP):
                        if touched[t][w0 + lw, b]:
                            ents.append([nent, b, False, False])
                            went.setdefault(b, []).append((len(cellw), len(ents) - 1))
                            nent += 1
                    cellw.append((t, w0 + lw, ents))
            for b, lst in went.items():
                wi, ei = lst[0]
                cellw[wi][2][ei][2] = True
                wi, ei = lst[-1]
                cellw[wi][2][ei][3] = True
            sg.append(cellw)
        sched.append(sg)

    # per-core slotcol [nent, 128]
    for c in range(NC):
        scol = np.full((nent, 128), -1.0, dtype=np.float32)
        for s in range(2):
            for g in range(NGRP):
                for (t, wg, ents) in sched[s][g]:
                    sb = per_core[c]["blk"][t][wg * 128:(wg + 1) * 128]
                    ss_ = per_core[c]["slot"][t][wg * 128:(wg + 1) * 128]
                    for (ent, b, _, _) in ents:
                        scol[ent] = np.where(sb == b, ss_, -1).astype(np.float32)
        per_core[c]["slotcol"] = scol

    return {"nw2": nw2, "wbase": wbase, "nwt": nwt, "sched": sched,
            "nent": nent, "per_core": per_core}


def host_prep(cfg, src, dst):
    src = np.asarray(src).astype(np.int64)
    dst = np.asarray(dst).astype(np.int64)
    l1 = _prep_l1(cfg, src, dst)
    l2 = _prep_l2(cfg, src, dst)
    return l1, l2


def build(cfg, l1, l2):
    N, IN, HID = cfg.N, cfg.IN, cfg.HID
    NW1 = l1["NW1"]
    nwt = [int(x) for x in l2["nwt"]]
    NENT2 = l2["nent"]

    nc = bacc.Bacc("TRN2", target_bir_lowering=False)

    xs_d = nc.dram_tensor("xs", [128, NW1 * WIN], FP8, kind="ExternalInput")
    idx_d = [nc.dram_tensor(f"idx{t}", [128, max(nwt[t] * 8, 8)], I16,
                            kind="ExternalInput") for t in range(4)]
    slot2_d = nc.dram_tensor("slot2", [128, NENT2], F32, kind="ExternalInput")
    segid_d = nc.dram_tensor("segid", [128, cfg.NBLK], F32, kind="ExternalInput")
    iota_d = nc.dram_tensor("iota128", [128, 128], BF16, kind="ExternalInput")
    iotas_d = nc.dram_tensor("iota_seg", [128, cfg.NSEGCH * 128], FP16, kind="ExternalInput")
    ident_d = nc.dram_tensor("ident", [128, 128], BF16, kind="ExternalInput")
    w1_d = nc.dram_tensor("W1s", [IN, HID], BF16, kind="ExternalInput")
    w2_d = nc.dram_tensor("W2", [HID, HID], BF16, kind="ExternalInput")
    b1_d = nc.dram_tensor("b1col", [128, 1], F32, kind="ExternalInput")
    b2_d = nc.dram_tensor("b2col", [128, 1], F32, kind="ExternalInput")
    ones_d = nc.dram_tensor("onescol", [128, 1], BF16, kind="ExternalInput")

    s2_sh = [nc.dram_tensor(f"s2_sh{s}", [cfg.HALF, HID], BF16) for s in range(2)]
    s2_full = [nc.dram_tensor(f"s2_full{s}", [cfg.NC * cfg.HALF, HID], BF16,
                              addr_space="Shared") for s in range(2)]
    out_d = nc.dram_tensor("pooled", [cfg.NSEGCH * 128, HID + 1], F32,
                           kind="ExternalOutput")

    KIN = IN // 128
    nw1 = [int(x) for x in l1["nw1"]]
    base1 = [int(x) for x in l1["base1"]]

    with tile.TileContext(nc) as tc, ExitStack() as ctx:
        const = ctx.enter_context(tc.tile_pool(name="const", bufs=1))
        xsp = ctx.enter_context(tc.tile_pool(name="xsp", bufs=4))
        sp = ctx.enter_context(tc.tile_pool(name="sp", bufs=4))
        hp = ctx.enter_context(tc.tile_pool(name="hp", bufs=3))
        saccp = ctx.enter_context(tc.tile_pool(name="saccp", bufs=1))
        normp = ctx.enter_context(tc.tile_pool(name="normp", bufs=1))

        # ---- constants ----
        iota128 = const.tile([128, 128], BF16)
        nc.sync.dma_start(iota128[:], iota_d[:])
        iotaseg = const.tile([128, cfg.NSEGCH * 128], FP16)
        nc.sync.dma_start(iotaseg[:], iotas_d[:])
        ident = const.tile([128, 128], BF16)
        nc.sync.dma_start(ident[:], ident_d[:])
        segid = const.tile([128, cfg.NBLK], F32)
        nc.sync.dma_start(segid[:], segid_d[:])
        w1_sb = [const.tile([128, HID], BF16, tag=f"w1_{k}", name=f"w1_{k}")
                 for k in range(KIN)]
        for k in range(KIN):
            nc.sync.dma_start(w1_sb[k][:], w1_d[k * 128:(k + 1) * 128, :])
        w2_sb = const.tile([128, HID], BF16)
        nc.sync.dma_start(w2_sb[:], w2_d[:])
        b1c = const.tile([128, 1], F32)
        nc.sync.dma_start(b1c[:], b1_d[:])
        b2c = const.tile([128, 1], F32)
        nc.sync.dma_start(b2c[:], b2_d[:])
        onesc = const.tile([128, 1], BF16)
        nc.sync.dma_start(onesc[:], ones_d[:])
        slot2 = const.tile([128, NENT2], F32)
        nc.sync.dma_start(slot2[:], slot2_d[:])

        # ================= layer 1 =================
        ctx_l1 = ctx.enter_context(ExitStack())
        ps_acc = ctx_l1.enter_context(tc.tile_pool(name="ps_acc", bufs=3, space="PSUM"))
        ps_h = ctx_l1.enter_context(tc.tile_pool(name="ps_h", bufs=1, space="PSUM"))
        ps_s2 = ctx_l1.enter_context(tc.tile_pool(name="ps_s2", bufs=1, space="PSUM"))
        ps_tr = ctx_l1.enter_context(tc.tile_pool(name="ps_tr", bufs=1, space="PSUM"))

        def l1_block(nb, acc):
            # acc: PSUM [128, 256] f32 node-major [slot, feat]
            ax = hp.tile([128, KIN * 128], BF16, tag="ax", name="ax")
            nc.scalar.activation(ax[:], acc[:], AF.Copy)
            h_ps = ps_h.tile([128, 128], F32, tag="hps", name="h_ps")
            for k in range(KIN):
                tr = ps_tr.tile([128, 128], BF16, tag="trk", name="tr")
                nc.tensor.transpose(tr[:], ax[:, k * 128:(k + 1) * 128], ident[:])
                xt = hp.tile([128, 128], BF16, tag="xt", name="xt")
                nc.scalar.activation(xt[:], tr[:], AF.Copy)
                nc.tensor.matmul(h_ps[:], w1_sb[k][:], xt[:],
                                 start=(k == 0), stop=(k == KIN - 1))
            h1t = hp.tile([128, 128], BF16, tag="h1t", name="h1t")
            nc.scalar.activation(h1t[:], h_ps[:], AF.Tanh, bias=b1c[:, 0:1])
            s2_ps = ps_s2.tile([128, 128], F32, tag="s2ps", name="s2_ps")
            nc.tensor.matmul(s2_ps[:], w2_sb[:], h1t[:], start=True, stop=True)
            s2t = hp.tile([128, 128], BF16, tag="s2t", name="s2t")
            nc.scalar.activation(s2t[:], s2_ps[:], AF.Copy)
            tr_ps = ps_tr.tile([128, 128], BF16, tag="trps")
            nc.tensor.transpose(tr_ps[:], s2t[:], ident[:])
            jb = nb % SB
            if jb == 0:
                s2n_cur[0] = hp.tile([128, SB * 128], BF16, tag="s2n", name="s2n")
            s2n = s2n_cur[0]
            nc.vector.tensor_copy(s2n[:, jb * 128:(jb + 1) * 128], tr_ps[:])
            if jb == SB - 1:
                st = nb // (cfg.NBLK // 2)
                r0 = ((nb - SB + 1) % (cfg.NBLK // 2)) * 128
                nc.sync.dma_start(
                    s2_sh[st][r0:r0 + SB * 128, :],
                    s2n[:].rearrange("p (j f) -> p j f", f=128))

        # stream chunks of WB windows; windows are block-major contiguous
        win_blk = []
        for b in range(cfg.NBLK):
            win_blk += [b] * nw1[b]
        acc_cur = {}
        s2n_cur = [None]
        for w0 in range(0, NW1, WB1):
            nwb = min(WB1, NW1 - w0)
            eb = xsp.tile([128, WB1 * WIN], FP8, tag="xs", name="xs")
            nc.sync.dma_start(eb[:, :nwb * WIN],
                              xs_d[:, w0 * WIN:(w0 + nwb) * WIN])
            for j in range(nwb):
                w = w0 + j
                b = win_blk[w]
                if b not in acc_cur:
                    acc_cur[b] = ps_acc.tile([128, KIN * 128], F32, tag="acc",
                                             name=f"acc")
                acc = acc_cur[b]
                first = (w == base1[b])
                last = (w == base1[b] + nw1[b] - 1)
                panel = eb[:, j * WIN + 256:j * WIN + 384]
                nc.tensor.matmul(acc[:], panel,
                                 eb[:, j * WIN:j * WIN + KIN * 128],
                                 start=first, stop=last)
                if last:
                    l1_block(b, acc)
                    del acc_cur[b]
                    # stage-0 exchange as soon as its half's blocks are done
                    if b == cfg.NBLK // 2 - 1:
                        nc.gpsimd.collective_compute(
                            "AllGather", ALU.bypass,
                            ins=[s2_sh[0].ap().opt()],
                            outs=[s2_full[0].ap().opt()],
                            replica_groups=[list(range(cfg.NC))])
        nc.gpsimd.collective_compute(
            "AllGather", ALU.bypass,
            ins=[s2_sh[1].ap().opt()], outs=[s2_full[1].ap().opt()],
            replica_groups=[list(range(cfg.NC))])

        # ================= layer 2 =================
        ctx_l1.close()
        ctx_l2 = ctx.enter_context(ExitStack())
        pacc_p = ctx_l2.enter_context(tc.tile_pool(name="pacc", bufs=2, space="PSUM"))
        ps_pool = ctx_l2.enter_context(tc.tile_pool(name="ps_pool", bufs=1, space="PSUM"))
        ps_ss = ctx_l2.enter_context(tc.tile_pool(name="ps_ss", bufs=1, space="PSUM"))
        ps_h2 = ctx_l2.enter_context(tc.tile_pool(name="ps_h2", bufs=2, space="PSUM"))
        ebp = [ctx_l2.enter_context(tc.tile_pool(name=f"eb2_{p}", bufs=4))
               for p in range(2)]
        idxp = ctx_l2.enter_context(tc.tile_pool(name="idxp", bufs=4))
        htp = ctx_l2.enter_context(tc.tile_pool(name="htp", bufs=4))

        sacc = saccp.tile([128, cfg.NBLK * 128], F32)
        ss_all = ps_ss.tile([128, cfg.NBLK], F32, name="ss_all")
        scale = normp.tile([128, cfg.NBLK], F32)
        pool_all = ps_pool.tile([128, cfg.NSEGCH * (HID + 1)], F32, name="pool_all")
        pool_ps = [pool_all[:, s * (HID + 1):(s + 1) * (HID + 1)]
                   for s in range(cfg.NSEGCH)]

        # per-t gather stream state: consumption cursor -> call issuing
        tbl_ap = [s2_full[t // 2][(t % 2) * cfg.SUBROWS:(t % 2 + 1) * cfg.SUBROWS, :]
                  for t in range(4)]
        cur_tile = [None] * 4
        cur_w0 = [0] * 4
        cur_it = [None] * 4
        cur_it_w0 = [-1] * 4

        def ensure_window(t, wg):
            """Return (tile, offset) for window wg of stream t, issuing its
            gather call (WB windows) and idx DMA (IB calls) if needed."""
            if cur_tile[t] is None or wg >= cur_w0[t] + WB:
                w0 = (wg // WB) * WB
                iw0 = (w0 // (WB * IB)) * (WB * IB)
                if cur_it_w0[t] != iw0:
                    niw = min(WB * IB, nwt[t] - iw0)
                    it = idxp.tile([128, WB * IB * 8], I16, tag="it", name="it")
                    nc.sync.dma_start(it[:, :niw * 8],
                                      idx_d[t][:, iw0 * 8:(iw0 + niw) * 8])
                    cur_it[t] = it
                    cur_it_w0[t] = iw0
                nwin = min(WB, nwt[t] - w0)
                jo = (w0 - iw0) * 8
                eb = ebp[t % 2].tile([128, WB * HID], BF16, tag=f"eb{t % 2}",
                                     name="eb")
                nc.gpsimd.dma_gather(
                    out_ap=eb[:, :nwin * HID].rearrange("p (n f) -> p n f", f=HID),
                    in_ap=tbl_ap[t],
                    idxs_ap=cur_it[t][:, jo:jo + nwin * 8],
                    num_idxs=nwin * 128,
                    num_idxs_reg=nwin * 128,
                    elem_size=HID)
                cur_tile[t] = eb
                cur_w0[t] = w0
            return cur_tile[t], wg - cur_w0[t]

        nbatch = cfg.NBLK // 2   # blocks per logmap half-batch
        h2_pend = []  # list of nb finalized since last flush
        h2_all = saccp.tile([128, cfg.NBLK * 128], BF16, name="h2_all")

        def flush_logmap():
            if not h2_pend:
                return
            b0 = h2_pend[0]
            nbk = len(h2_pend)
            ss = ss_all[:, b0:b0 + nbk]
            na = normp.tile([128, nbatch], F32, tag="na")
            nc.vector.tensor_scalar_max(na[:, :nbk], ss, MIN_SS)
            nrm = normp.tile([128, nbatch], F32, tag="nrm")
            nc.scalar.activation(nrm[:, :nbk], na[:, :nbk], AF.Sqrt)
            ncl = normp.tile([128, nbatch], F32, tag="ncl")
            nc.vector.tensor_scalar_min(ncl[:, :nbk], nrm[:, :nbk], MAXNORM)
            om = normp.tile([128, nbatch], F32, tag="om")
            nc.vector.tensor_scalar(om[:, :nbk], ncl[:, :nbk], -1.0, 1.0,
                                    ALU.mult, ALU.add)
            op_ = normp.tile([128, nbatch], F32, tag="op")
            nc.vector.tensor_scalar_add(op_[:, :nbk], ncl[:, :nbk], 1.0)
            rc = normp.tile([128, nbatch], F32, tag="rc")
            nc.vector.reciprocal(rc[:, :nbk], om[:, :nbk])
            rat = normp.tile([128, nbatch], F32, tag="rat")
            nc.vector.tensor_mul(rat[:, :nbk], op_[:, :nbk], rc[:, :nbk])
            lg = normp.tile([128, nbatch], F32, tag="lg")
            nc.scalar.activation(lg[:, :nbk], rat[:, :nbk], AF.Ln)
            rcn = normp.tile([128, nbatch], F32, tag="rcn")
            nc.vector.reciprocal(rcn[:, :nbk], nrm[:, :nbk])
            nc.vector.tensor_mul(rcn[:, :nbk], rcn[:, :nbk], lg[:, :nbk])
            nc.vector.tensor_scalar_mul(scale[:, b0:b0 + nbk], rcn[:, :nbk], 0.5)
            # scale ready: finish blocks
            for nb in h2_pend:
                ht = htp.tile([128, HID + 1], BF16, tag="ht", name="ht")
                nc.vector.tensor_scalar(ht[:, :HID],
                                        h2_all[:, nb * 128:(nb + 1) * 128],
                                        scale[:, nb:nb + 1], None, ALU.mult)
                nc.vector.memset(ht[:, HID:HID + 1], 1.0)
                sg = sp.tile([128, cfg.NSEGCH * 128], FP16, tag="sg")
                nc.vector.tensor_scalar(sg[:], iotaseg[:], segid[:, nb:nb + 1],
                                        None, ALU.is_equal)
                for s in range(cfg.NSEGCH):
                    nc.tensor.matmul(pool_ps[s], sg[:, s * 128:(s + 1) * 128],
                                     ht[:], start=(nb == 0),
                                     stop=(nb == cfg.NBLK - 1))
            h2_pend.clear()

        for stage in range(2):
            for g in range(cfg.NGRP):
                cellw = l2["sched"][stage][g]
                pacc = pacc_p.tile([128, GRP * 128], F32, tag="pacc",
                                   name="pacc")
                for (t, wg, ents) in cellw:
                    eb, joff = ensure_window(t, wg)
                    for (ent, b, st_f, sp_f) in ents:
                        m = sp.tile([128, 128], BF16, tag="m2", name="m2")
                        nc.vector.tensor_scalar(m[:], iota128[:],
                                                slot2[:, ent:ent + 1],
                                                None, ALU.is_equal)
                        nc.tensor.matmul(pacc[:, b * 128:(b + 1) * 128],
                                         eb[:, joff * HID:(joff + 1) * HID],
                                         m[:], start=st_f, stop=sp_f)
                for b in range(GRP):
                    nb = g * GRP + b
                    pb = pacc[:, b * 128:(b + 1) * 128]
                    sl = sacc[:, nb * 128:(nb + 1) * 128]
                    if stage == 0:
                        nc.vector.tensor_copy(sl, pb)
                    else:
                        h2pre = hp.tile([128, 128], F32, tag="h2pre", name="h2pre")
                        nc.vector.scalar_tensor_tensor(
                            h2pre[:], pb, b2c[:, 0:1], sl,
                            ALU.add, ALU.add)
                        h2t = hp.tile([128, 128], BF16, tag="h2t")
                        nc.scalar.activation(h2t[:], h2pre[:], AF.Tanh)
                        sq = hp.tile([128, 128], BF16, tag="sq")
                        nc.scalar.activation(sq[:], h2t[:], AF.Square)
                        nc.tensor.matmul(ss_all[:, nb:nb + 1], sq[:], onesc[:],
                                         start=True, stop=True)
                        h2tr = ps_h2.tile([128, 128], BF16, tag="h2b",
                                          name="h2tr")
                        nc.tensor.transpose(h2tr[:], h2t[:], ident[:])
                        nc.vector.tensor_copy(
                            h2_all[:, nb * 128:(nb + 1) * 128], h2tr[:])
                        h2_pend.append(nb)
                        if len(h2_pend) >= nbatch:
                            flush_logmap()
        flush_logmap()

        for s in range(cfg.NSEGCH):
            po = htp.tile([128, HID + 1], F32, tag="po")
            nc.vector.tensor_copy(po[:], pool_ps[s])
            nc.sync.dma_start(out_d[s * 128:(s + 1) * 128, :], po[:])

    nc.compile()
    return nc


def host_inputs(cfg, x, seg_ids, W1, b1, W2, b2, l1, l2):
    N, IN, HID = cfg.N, cfg.IN, cfg.HID
    NW1 = l1["NW1"]
    x8 = np.ascontiguousarray((np.asarray(x, np.float32) * XSCALE).astype(NP_FP8))
    iota128 = np.tile(np.arange(128, dtype=np.float32), (128, 1)).astype(NP_BF16)
    iotaseg = np.tile(np.arange(cfg.NSEGCH * 128, dtype=np.float32),
                      (128, 1)).astype(np.float16)
    ident = np.eye(128, dtype=np.float32).astype(NP_BF16)
    w1 = np.ascontiguousarray(
        (np.asarray(W1, np.float32) / XSCALE).astype(NP_BF16))
    w2 = np.ascontiguousarray(np.asarray(W2, np.float32).astype(NP_BF16))
    b1c = np.asarray(b1, np.float32).reshape(128, 1)
    b2c = np.asarray(b2, np.float32).reshape(128, 1)
    ones = np.ones((128, 1), np.float32).astype(NP_BF16)
    seg = np.asarray(seg_ids, np.float32)
    eye128 = np.eye(128, dtype=np.float32)

    maps = []
    for c in range(cfg.NC):
        pc1 = l1["per_core"][c]
        sidx = pc1["srcidx"]
        rows = x8[np.maximum(sidx, 0)]                     # [NW1*128, 256]
        rows[sidx < 0] = 0
        pslot = pc1["slot"]
        panels = np.zeros((NW1 * 128, 128), dtype=NP_FP8)
        valid = pslot >= 0
        panels[np.nonzero(valid)[0], pslot[valid]] = 1.0
        xs = np.concatenate([rows, panels], axis=1)        # [NW1*128, 384]
        xs = np.ascontiguousarray(
            xs.reshape(NW1, 128, WIN).transpose(1, 0, 2).reshape(128, NW1 * WIN))

        pc2 = l2["per_core"][c]
        idxs = {}
        for t in range(4):
            ids = pc2["idx"][t]
            iw = ids.astype(np.int16).reshape(-1, 16).T
            iw = np.tile(iw, (8, 1)).copy()
            idxs[f"idx{t}"] = iw.astype(np.int16)
            if iw.shape[1] == 0:
                idxs[f"idx{t}"] = np.zeros((128, 8), dtype=np.int16)

        segc = seg[c * cfg.SHARD:(c + 1) * cfg.SHARD].reshape(cfg.NBLK, 128).T
        maps.append({
            "xs": xs,
            **idxs,
            "slot2": np.ascontiguousarray(pc2["slotcol"].T),
            "segid": np.ascontiguousarray(segc.astype(np.float32)),
            "iota128": iota128,
            "iota_seg": iotaseg,
            "ident": ident,
            "W1s": w1,
            "W2": w2,
            "b1col": b1c,
            "b2col": b2c,
            "onescol": ones,
        })
    return maps


def host_epilogue(cfg, partials, batch_size, max_comments):
    acc = np.zeros_like(partials[0], dtype=np.float64)
    for p in partials:
        acc += p.astype(np.float64)
    acc = acc.astype(np.float32)
    nseg = cfg.NSEG
    sums = acc[:nseg, :cfg.HID]
    counts = acc[:nseg, cfg.HID]
    agg = sums / np.maximum(counts, 1.0)[:, None]
    ss = np.maximum(np.sum(agg * agg, axis=1), MIN_SS).astype(np.float32)
    norm = np.sqrt(ss)
    y = agg * (np.tanh(norm) / norm)[:, None]
    ssy = np.maximum(np.sum(y * y, axis=1), MIN_SS).astype(np.float32)
    ny = np.sqrt(ssy)
    f = np.where(ny > MAXNORM, MAXNORM / ny, 1.0).astype(np.float32)
    y = y * f[:, None]
    return y.reshape(int(batch_size), int(max_comments), cfg.HID)


# ====================================================================
# Harness entry point
# ====================================================================

_CACHE = {}


def kernel(x, src, dst, seg_ids, W1, b1, W2, b2, batch_size, max_comments):
    """Full-input GNN ComEnc kernel on 8 Trainium2 NeuronCores."""
    from concourse.bass_utils import run_bass_kernel_spmd

    x = np.asarray(x, dtype=np.float32)
    src = np.asarray(src).astype(np.int64)
    dst = np.asarray(dst).astype(np.int64)
    seg_ids = np.asarray(seg_ids).astype(np.int64)
    W1 = np.asarray(W1, dtype=np.float32)
    b1 = np.asarray(b1, dtype=np.float32)
    W2 = np.asarray(W2, dtype=np.float32)
    b2 = np.asarray(b2, dtype=np.float32)
    bs = int(np.asarray(batch_size))
    mc = int(np.asarray(max_comments))

    n_nodes, in_dim = x.shape
    hid = W1.shape[1]
    nseg = bs * mc
    n_cores = 8

    cfg = Cfg(n_nodes, in_dim, hid, nseg, n_cores)
    l1, l2 = host_prep(cfg, src, dst)

    key = (n_nodes, in_dim, hid, nseg, l1["NW1"], l2["nent"],
           tuple(int(v) for v in l2["nwt"]))
    if key in _CACHE:
        nc = _CACHE[key]
    else:
        nc = build(cfg, l1, l2)
        _CACHE.clear()
        _CACHE[key] = nc

    maps = host_inputs(cfg, x, seg_ids, W1, b1, W2, b2, l1, l2)
    res = run_bass_kernel_spmd(nc, maps, core_ids=list(range(n_cores)))
    partials = [r["pooled"] for r in res.results]
    out = host_epilogue(cfg, partials, bs, mc)
    return np.ascontiguousarray(out.astype(np.float32))
